# revision 8
# baseline (speedup 1.0000x reference)
"""Edge-decoder (GNN link prediction) kernel for 8 Trainium2 NeuronCores.

Computes logits[e] = sum_d x[src[e], d] * x[tar[e], d] for 640K edges
(pos then neg), node table x [100000, 128] f32.

Strategy (edges sharded contiguously across 8 cores, x replicated):
per core, its 80000 edges are bucketed on the host by the pair
(src_node // 25000, tar_node // 25000) -> 16 buckets, so that every node
index used inside a bucket fits in an int16 offset (< 25000) relative to a
statically-known 25000-row chunk base of x. Each bucket's src and tar rows
are then fetched with ONE dma_gather custom instruction per side, instead
of the 625 serialized 128-row indirect DMAs the original version used.

dma_gather throughput here is bounded by Q7 descriptor GENERATION
(~7.6 ns/descriptor per SWDGE queue-pair), not by the SDMA drain, so the
kernel claims all four SWDGE queues (8 Q7 cores = 4 tx/rx pairs): bucket
b's src/tar gathers ride queues (2b)%4 / (2b)%4+1, alternating queue
pairs between consecutive buckets. Rows are gathered in f32: 512B
descriptors drain at SDMA line rate while 256B (bf16) descriptors pay a
~3x read-modify-write penalty, so f32 is net faster despite double bytes.
Gather tiles are triple-buffered so the DVE stage never stalls the Q7
queues; all 32 idx tiles are loaded in 2 up-front DMAs and the 16 logit
columns accumulate in one SBUF tile flushed by a single final DMA.

Gathered tiles land as [128, G, 128] f32 (edge i -> partition i%128,
group i//128); a DVE tensor_tensor multiply (bf16 product output) and a
tensor_reduce over the feature axis produce one f32 logit per edge.
Buckets are padded with -1 indices (trimmed by the Q7 at runtime, so DGE
and SDMA cost track the true bucket sizes) to a static capacity; the host
inverse-permutes the per-bucket logits back to edge order.
"""

import numpy as np

N_NODES = 100000
D = 128
E_TOTAL = 640000
N_CORES = 8
N_CHUNK = 4
CHUNK = N_NODES // N_CHUNK  # 25000 < 32768 so offsets fit int16
NB = N_CHUNK * N_CHUNK  # 16 buckets

E_CORE = E_TOTAL // N_CORES  # 80000
CAP_DEFAULT = 5248  # 41 groups of 128; bucket mean 5000, sigma ~68

_cached = {}


def build(cap, dyn_repeat=False):
    from concourse import bacc, mybir, tile

    G = cap // 128
    ci = cap // 16
    nc = bacc.Bacc(
        "TRN2",
        target_bir_lowering=False,
        debug=False,
        num_devices=N_CORES,
        num_swdge_queues=4,
    )
    x = nc.dram_tensor(
        "x", [N_NODES, D], mybir.dt.float32, kind="ExternalInput"
    ).ap()
    src_idx = nc.dram_tensor(
        "src_idx", [128, NB, ci], mybir.dt.int16, kind="ExternalInput"
    ).ap()
    tar_idx = nc.dram_tensor(
        "tar_idx", [128, NB, ci], mybir.dt.int16, kind="ExternalInput"
    ).ap()
    counts = nc.dram_tensor(
        "counts", [1, NB], mybir.dt.int32, kind="ExternalInput"
    ).ap()
    if dyn_repeat:
        reps = nc.dram_tensor(
            "reps", [1, 1], mybir.dt.int32, kind="ExternalInput"
        ).ap()
    logits = nc.dram_tensor(
        "logits", [128, NB, G, 1], mybir.dt.float32, kind="ExternalOutput"
    ).ap()

    with tile.TileContext(nc) as tc:
        with tc.tile_pool(name="misc", bufs=1) as miscp, tc.tile_pool(
            name="idx", bufs=2
        ) as idxp, tc.tile_pool(name="gat", bufs=3) as gatp, tc.tile_pool(
            name="prod", bufs=2
        ) as prodp, tc.tile_pool(name="lg", bufs=2) as lgp:
            cnt_t = miscp.tile([1, NB], mybir.dt.int32)
            nc.sync.dma_start(out=cnt_t[:], in_=counts)
            if dyn_repeat:
                reps_t = miscp.tile([1, 1], mybir.dt.int32)
                nc.sync.dma_start(out=reps_t[:], in_=reps)

            def workload():
                si_t = idxp.tile([128, NB, ci], mybir.dt.int16, tag="si")
                nc.scalar.dma_start(out=si_t[:], in_=src_idx)
                ti_t = idxp.tile([128, NB, ci], mybir.dt.int16, tag="ti")
                nc.scalar.dma_start(out=ti_t[:], in_=tar_idx)
                lg_t = lgp.tile([128, NB, G, 1], mybir.dt.float32, tag="lg")
                for b in range(NB):
                    bs, bt = b // N_CHUNK, b % N_CHUNK
                    n_reg = nc.values_load(
                        cnt_t[0:1, b : b + 1],
                        engines=(mybir.EngineType.Pool,),
                        min_val=1,
                        max_val=cap,
                        skip_runtime_bounds_check=True,
                    )
                    s_t = gatp.tile([128, G, D], mybir.dt.float32, tag="s")
                    nc.gpsimd.dma_gather(
                        s_t[:],
                        x[bs * CHUNK : (bs + 1) * CHUNK, :],
                        si_t[:, b, :],
                        cap,
                        n_reg,
                        D,
                        # single_packet coalesces each engine's stream into
                        # one packet; >64 descs/engine (1024 idxs) is out of
                        # spec for the SDMA packet format
                        single_packet=False,
                        queue_num=(2 * b) % 4,
                    )
                    t_t = gatp.tile([128, G, D], mybir.dt.float32, tag="t")
                    nc.gpsimd.dma_gather(
                        t_t[:],
                        x[bt * CHUNK : (bt + 1) * CHUNK, :],
                        ti_t[:, b, :],
                        cap,
                        n_reg,
                        D,
                        single_packet=False,
                        queue_num=(2 * b) % 4 + 1,
                    )
                    p_t = prodp.tile([128, G, D], mybir.dt.bfloat16, tag="p")
                    nc.vector.tensor_tensor(
                        out=p_t[:], in0=s_t[:], in1=t_t[:], op=mybir.AluOpType.mult
                    )
                    nc.vector.tensor_reduce(
                        out=lg_t[:, b],
                        in_=p_t[:],
                        axis=mybir.AxisListType.X,
                        op=mybir.AluOpType.add,
                    )
                nc.scalar.dma_start(out=logits, in_=lg_t[:])

            if dyn_repeat:
                r_val = nc.values_load(
                    reps_t[0:1, 0:1],
                    min_val=1,
                    max_val=1000,
                    skip_runtime_bounds_check=True,
                )
                with tc.For_i(0, r_val):
                    workload()
            else:
                workload()
    nc.compile()
    return nc


def _get_nc(cap, dyn_repeat=False):
    key = (cap, dyn_repeat)
    if key not in _cached:
        _cached[key] = build(cap, dyn_repeat)
    return _cached[key]


def host_prepare(x, src, tar, cap=None):
    """Bucket/pad per core. Returns (in_maps, restore, cap) where restore is
    a list of (order, counts) to reassemble per-core edge-ordered logits."""
    x_f = np.ascontiguousarray(np.asarray(x, np.float32))
    max_n = 0
    per_core = []
    for c in range(N_CORES):
        s = src[c * E_CORE : (c + 1) * E_CORE]
        t = tar[c * E_CORE : (c + 1) * E_CORE]
        b = (s // CHUNK) * N_CHUNK + (t // CHUNK)
        order = np.argsort(b, kind="stable")
        cnts = np.bincount(b, minlength=NB).astype(np.int32)
        per_core.append((s, t, b, order, cnts))
        max_n = max(max_n, int(cnts.max()))
    if cap is None:
        cap = max(CAP_DEFAULT, ((max_n + 127) // 128) * 128)
    assert max_n <= cap

    in_maps, restore = [], []
    for s, t, b, order, cnts in per_core:
        s_off = (s - (s // CHUNK) * CHUNK).astype(np.int16)
        t_off = (t - (t // CHUNK) * CHUNK).astype(np.int16)
        sflat = np.full((NB, cap), -1, np.int16)
        tflat = np.full((NB, cap), -1, np.int16)
        pos = 0
        counts_eff = cnts.copy()
        for bi in range(NB):
            n = int(cnts[bi])
            sel = order[pos : pos + n]
            pos += n
            sflat[bi, :n] = s_off[sel]
            tflat[bi, :n] = t_off[sel]
            if n == 0:  # Q7/interp need >= 1 valid index
                sflat[bi, 0] = 0
                tflat[bi, 0] = 0
                counts_eff[bi] = 1
        # logical idx i of bucket b -> partition i%16, col i//16,
        # replicated over the 8 partition groups; layout [128, NB, cap//16]
        sw = np.tile(
            sflat.reshape(NB, cap // 16, 16).transpose(2, 0, 1), (8, 1, 1)
        )
        tw = np.tile(
            tflat.reshape(NB, cap // 16, 16).transpose(2, 0, 1), (8, 1, 1)
        )
        in_maps.append(
            {
                "x": x_f,
                "src_idx": np.ascontiguousarray(sw),
                "tar_idx": np.ascontiguousarray(tw),
                "counts": counts_eff[None, :],
            }
        )
        restore.append((order, cnts))
    return in_maps, restore, cap


def assemble(results, restore):
    out = np.empty((E_TOTAL, 1), np.float32)
    for c in range(N_CORES):
        lg = np.asarray(results[c]["logits"])  # [128, NB, G, 1]
        order, cnts = restore[c]
        vals = np.empty(E_CORE, np.float32)
        pos = 0
        flat = lg[:, :, :, 0].transpose(1, 2, 0).reshape(NB, -1)
        for bi in range(NB):
            n = int(cnts[bi])
            vals[pos : pos + n] = flat[bi, :n]
            pos += n
        oc = np.empty(E_CORE, np.float32)
        oc[restore[c][0]] = vals
        out[c * E_CORE : (c + 1) * E_CORE, 0] = oc
    return out


def kernel(x, pos_edge_index, neg_edge_index):
    from concourse.bass_utils import run_bass_kernel_spmd

    src = np.concatenate(
        [np.asarray(pos_edge_index[0]), np.asarray(neg_edge_index[0])]
    ).astype(np.int32)
    tar = np.concatenate(
        [np.asarray(pos_edge_index[1]), np.asarray(neg_edge_index[1])]
    ).astype(np.int32)

    in_maps, restore, cap = host_prepare(np.asarray(x), src, tar)
    nc = _get_nc(cap)
    res = run_bass_kernel_spmd(nc, in_maps, core_ids=list(range(N_CORES)))
    return assemble(res.results, restore)


# revision 9
# speedup vs baseline: 1.1456x; 1.1456x over previous
"""Edge-decoder (GNN link prediction) kernel for 8 Trainium2 NeuronCores.

Computes logits[e] = sum_d x[src[e], d] * x[tar[e], d] for 640K edges
(pos then neg), node table x [100000, 128] f32.

Strategy (edges sharded contiguously across 8 cores, x replicated):
per core, its 80000 edges are bucketed on the host by the pair
(src_node // 25000, tar_node // 25000) -> 16 buckets, so that every node
index used inside a bucket fits in an int16 offset (< 25000) relative to a
statically-known 25000-row chunk base of x. Each bucket's src and tar rows
are then fetched with ONE dma_gather custom instruction per side, instead
of the 625 serialized 128-row indirect DMAs the original version used.

dma_gather throughput here is bounded by Q7 descriptor GENERATION
(~7.6 ns/descriptor per SWDGE queue-pair), not by the SDMA drain, so the
kernel claims all four SWDGE queues (8 Q7 cores = 4 tx/rx pairs) and
processes buckets in pairs: src/tar gathers of bucket 2k ride queues 0/1
while bucket 2k+1 rides queues 2/3. Rows are gathered in f32: 512B
descriptors drain at SDMA line rate while 256B (bf16) descriptors pay a
~3x read-modify-write penalty, so f32 is net faster despite double bytes.

Gathered tiles land as [128, G, 128] f32 (edge i -> partition i%128,
group i//128); a DVE tensor_tensor multiply (bf16 product output) and a
tensor_reduce over the feature axis produce one f32 logit per edge.
Buckets are padded with -1 indices (trimmed by the Q7 at runtime, so DGE
and SDMA cost track the true bucket sizes) to a static capacity; the host
inverse-permutes the per-bucket logits back to edge order.
"""

import numpy as np

N_NODES = 100000
D = 128
E_TOTAL = 640000
N_CORES = 8
N_CHUNK = 4
CHUNK = N_NODES // N_CHUNK  # 25000 < 32768 so offsets fit int16
NB = N_CHUNK * N_CHUNK  # 16 buckets

E_CORE = E_TOTAL // N_CORES  # 80000
CAP_DEFAULT = 5248  # 41 groups of 128; bucket mean 5000, sigma ~68

_cached = {}


def build(cap, dyn_repeat=False):
    from concourse import bacc, mybir, tile

    G = cap // 128
    nc = bacc.Bacc(
        "TRN2",
        target_bir_lowering=False,
        debug=False,
        num_devices=N_CORES,
        num_swdge_queues=4,
    )
    x = nc.dram_tensor(
        "x", [N_NODES, D], mybir.dt.float32, kind="ExternalInput"
    ).ap()
    src_idx = nc.dram_tensor(
        "src_idx", [NB, 128, cap // 16], mybir.dt.int16, kind="ExternalInput"
    ).ap()
    tar_idx = nc.dram_tensor(
        "tar_idx", [NB, 128, cap // 16], mybir.dt.int16, kind="ExternalInput"
    ).ap()
    counts = nc.dram_tensor(
        "counts", [1, NB], mybir.dt.int32, kind="ExternalInput"
    ).ap()
    if dyn_repeat:
        reps = nc.dram_tensor(
            "reps", [1, 1], mybir.dt.int32, kind="ExternalInput"
        ).ap()
    logits = nc.dram_tensor(
        "logits", [NB, 128, G, 1], mybir.dt.float32, kind="ExternalOutput"
    ).ap()

    with tile.TileContext(nc) as tc:
        with tc.tile_pool(name="misc", bufs=1) as miscp, tc.tile_pool(
            name="idx", bufs=4
        ) as idxp, tc.tile_pool(name="gat", bufs=2) as gatp, tc.tile_pool(
            name="prod", bufs=1
        ) as prodp, tc.tile_pool(name="lg", bufs=3) as lgp:
            cnt_t = miscp.tile([1, NB], mybir.dt.int32)
            nc.sync.dma_start(out=cnt_t[:], in_=counts)
            if dyn_repeat:
                reps_t = miscp.tile([1, 1], mybir.dt.int32)
                nc.sync.dma_start(out=reps_t[:], in_=reps)

            def bucket_pair(b0):
                tiles = {}
                for j, b in enumerate((b0, b0 + 1)):
                    bs, bt = b // N_CHUNK, b % N_CHUNK
                    si_t = idxp.tile(
                        [128, cap // 16], mybir.dt.int16, tag=f"si{j}"
                    )
                    nc.scalar.dma_start(out=si_t[:], in_=src_idx[b])
                    ti_t = idxp.tile(
                        [128, cap // 16], mybir.dt.int16, tag=f"ti{j}"
                    )
                    nc.scalar.dma_start(out=ti_t[:], in_=tar_idx[b])
                    n_reg = nc.values_load(
                        cnt_t[0:1, b : b + 1],
                        engines=(mybir.EngineType.Pool,),
                        min_val=1,
                        max_val=cap,
                        skip_runtime_bounds_check=True,
                    )
                    s_t = gatp.tile([128, G, D], mybir.dt.float32, tag=f"s{j}")
                    nc.gpsimd.dma_gather(
                        s_t[:],
                        x[bs * CHUNK : (bs + 1) * CHUNK, :],
                        si_t[:],
                        cap,
                        n_reg,
                        D,
                        # single_packet coalesces each engine's stream into
                        # one packet; >64 descs/engine (1024 idxs) is out of
                        # spec for the SDMA packet format
                        single_packet=False,
                        queue_num=2 * j,
                    )
                    t_t = gatp.tile([128, G, D], mybir.dt.float32, tag=f"t{j}")
                    nc.gpsimd.dma_gather(
                        t_t[:],
                        x[bt * CHUNK : (bt + 1) * CHUNK, :],
                        ti_t[:],
                        cap,
                        n_reg,
                        D,
                        single_packet=False,
                        queue_num=2 * j + 1,
                    )
                    tiles[b] = (s_t, t_t)
                for j, b in enumerate((b0, b0 + 1)):
                    s_t, t_t = tiles[b]
                    p_t = prodp.tile([128, G, D], mybir.dt.bfloat16, tag=f"p{j}")
                    nc.vector.tensor_tensor(
                        out=p_t[:], in0=s_t[:], in1=t_t[:], op=mybir.AluOpType.mult
                    )
                    lg_t = lgp.tile([128, G, 1], mybir.dt.float32, tag="lg")
                    nc.vector.tensor_reduce(
                        out=lg_t[:],
                        in_=p_t[:],
                        axis=mybir.AxisListType.X,
                        op=mybir.AluOpType.add,
                    )
                    nc.scalar.dma_start(out=logits[b], in_=lg_t[:])

            if dyn_repeat:
                r_val = nc.values_load(
                    reps_t[0:1, 0:1],
                    min_val=1,
                    max_val=1000,
                    skip_runtime_bounds_check=True,
                )
                with tc.For_i(0, r_val):
                    for b0 in range(0, NB, 2):
                        bucket_pair(b0)
            else:
                for b0 in range(0, NB, 2):
                    bucket_pair(b0)
    nc.compile()
    return nc


def _get_nc(cap, dyn_repeat=False):
    key = (cap, dyn_repeat)
    if key not in _cached:
        _cached[key] = build(cap, dyn_repeat)
    return _cached[key]


def host_prepare(x, src, tar, cap=None):
    """Bucket/pad per core. Returns (in_maps, restore, cap) where restore is
    a list of (order, counts) to reassemble per-core edge-ordered logits."""
    x_f = np.ascontiguousarray(np.asarray(x, np.float32))
    max_n = 0
    per_core = []
    for c in range(N_CORES):
        s = src[c * E_CORE : (c + 1) * E_CORE]
        t = tar[c * E_CORE : (c + 1) * E_CORE]
        b = (s // CHUNK) * N_CHUNK + (t // CHUNK)
        order = np.argsort(b, kind="stable")
        cnts = np.bincount(b, minlength=NB).astype(np.int32)
        per_core.append((s, t, b, order, cnts))
        max_n = max(max_n, int(cnts.max()))
    if cap is None:
        cap = max(CAP_DEFAULT, ((max_n + 127) // 128) * 128)
    assert max_n <= cap

    in_maps, restore = [], []
    for s, t, b, order, cnts in per_core:
        s_off = (s - (s // CHUNK) * CHUNK).astype(np.int16)
        t_off = (t - (t // CHUNK) * CHUNK).astype(np.int16)
        sflat = np.full((NB, cap), -1, np.int16)
        tflat = np.full((NB, cap), -1, np.int16)
        pos = 0
        counts_eff = cnts.copy()
        for bi in range(NB):
            n = int(cnts[bi])
            sel = order[pos : pos + n]
            pos += n
            sflat[bi, :n] = s_off[sel]
            tflat[bi, :n] = t_off[sel]
            if n == 0:  # Q7/interp need >= 1 valid index
                sflat[bi, 0] = 0
                tflat[bi, 0] = 0
                counts_eff[bi] = 1
        # logical idx i -> partition i%16, col i//16; replicate over 8 groups
        sw = np.tile(
            sflat.reshape(NB, cap // 16, 16).transpose(0, 2, 1), (1, 8, 1)
        )
        tw = np.tile(
            tflat.reshape(NB, cap // 16, 16).transpose(0, 2, 1), (1, 8, 1)
        )
        in_maps.append(
            {
                "x": x_f,
                "src_idx": np.ascontiguousarray(sw),
                "tar_idx": np.ascontiguousarray(tw),
                "counts": counts_eff[None, :],
            }
        )
        restore.append((order, cnts))
    return in_maps, restore, cap


def assemble(results, restore):
    out = np.empty((E_TOTAL, 1), np.float32)
    for c in range(N_CORES):
        lg = np.asarray(results[c]["logits"])  # [NB, 128, G, 1]
        order, cnts = restore[c]
        vals = np.empty(E_CORE, np.float32)
        pos = 0
        flat = lg[:, :, :, 0].transpose(0, 2, 1).reshape(NB, -1)
        for bi in range(NB):
            n = int(cnts[bi])
            vals[pos : pos + n] = flat[bi, :n]
            pos += n
        oc = np.empty(E_CORE, np.float32)
        oc[restore[c][0]] = vals
        out[c * E_CORE : (c + 1) * E_CORE, 0] = oc
    return out


def kernel(x, pos_edge_index, neg_edge_index):
    from concourse.bass_utils import run_bass_kernel_spmd

    src = np.concatenate(
        [np.asarray(pos_edge_index[0]), np.asarray(neg_edge_index[0])]
    ).astype(np.int32)
    tar = np.concatenate(
        [np.asarray(pos_edge_index[1]), np.asarray(neg_edge_index[1])]
    ).astype(np.int32)

    in_maps, restore, cap = host_prepare(np.asarray(x), src, tar)
    nc = _get_nc(cap)
    res = run_bass_kernel_spmd(nc, in_maps, core_ids=list(range(N_CORES)))
    return assemble(res.results, restore)
